# revision 1
# baseline (speedup 1.0000x reference)
"""Trainium2 Bass kernel for segmented per-(d,k) 1D conv (PartiallyUnsharedConv1d).

Problem (hardcoded):
  x      [B=4, D=32, K=8, CI=2, L=4096] f32
  weight [D, K, CO=2, CI, S=8, 1, NB=15] f32
  bias   [D, K, CO, S, 1] f32
  out    [B, D, K, CO, L] f32

  out[b,d,k,o,l] = sum_{i,f} weight[d,k,o,i,seg(l),0,f] * xpad[b,d,k,i,l+f]
                   + bias[d,k,o,seg(l),0]
  where xpad is x zero-padded by P=7 on both ends of l, seg(l) assigns l to one
  of 8 contiguous segments (7x499 + 603).

Sharding: 8 cores = 4 d-groups x 2 b-groups. Each core owns 64 (d,k) pairs and
2 batch entries. Per core all 128 SBUF partitions are filled with (dk, i) rows;
a block-diagonal (64 blocks of 2x2) stationary matrix per (segment, tap) turns
the whole per-core conv into 15 PSUM-accumulated matmuls per output tile, with
the tap shift realized as a shifted SBUF slice of the padded x. No cross-core
communication.
"""

import numpy as np

# problem dims
B, D, K, CI, CO, L, NB, P, S = 4, 32, 8, 2, 2, 4096, 15, 7, 8
LP = L + 2 * P  # 4110

# segment layout (replicates reference _segment_ids)
_rough = LP // S
SEG_LENS = [_rough - 2 * P] * (S - 1)  # 499 x 7
SEG_LENS.append(L - sum(SEG_LENS))  # 603
SEG_STARTS = np.concatenate([[0], np.cumsum(SEG_LENS)[:-1]]).tolist()

# sharding
N_CORES = 8
DG, BG = 4, 2  # d-groups x b-groups
D_PER = D // DG  # 8
B_PER = B // BG  # 2
DK = D_PER * K  # 64 (d,k) pairs per core
NPART = 128
MAX_N = 512  # fp32 PSUM bank / moving-operand limit

_prog_cache = {}

# x rows are over-allocated past LP so that even-padded matmuls may read a few
# columns past the real data (zeros); fp32r requires an even moving-dim count.
LXP = LP + 8  # 4118


def _subtiles(s):
    """(t0, n_use, n_mm) tiles for segment s: n_use real outputs, n_mm the
    (even, >=256 where possible, <=MAX_N) matmul free-dim actually computed."""
    start, ln = SEG_STARTS[s], SEG_LENS[s]
    tiles = []
    if ln <= MAX_N:
        spans = [(start, ln)]
    else:
        h = ln // 2
        spans = [(start, h), (start + h, ln - h)]
    for t0, n_use in spans:
        n_mm = n_use + (n_use % 2)
        tiles.append((t0, n_use, n_mm))
    return tiles


# meta tensor per-partition layout (fp32 elements):
#   [0:240)    compact weights, col = s*30 + f*2 + o, row = (dk, i)
#   [240:368)  block-diag mask: mask[p, m] = (p//2 == m//2)
#   [368:376)  bias, col = s, row = (dk, o)
OFF_MASK = S * NB * CO  # 240
OFF_BIAS = OFF_MASK + NPART  # 368
TOT_META = OFF_BIAS + S  # 376


def _build_program(compute_dt="float32r", repeat=1, loop_n=None, tile4=False):
    import contextlib

    import concourse.mybir as mybir
    import concourse.tile as tile
    from concourse import bacc

    cdt = getattr(mybir.dt, compute_dt)
    f32 = mybir.dt.float32

    nc = bacc.Bacc("TRN2", target_bir_lowering=False, debug=False)

    meta_d = nc.dram_tensor("meta", [NPART, TOT_META], f32, kind="ExternalInput").ap()
    xa_d = nc.dram_tensor("xa", [NPART, LXP], cdt, kind="ExternalInput").ap()
    xb_d = nc.dram_tensor("xb", [NPART, LXP], cdt, kind="ExternalInput").ap()
    out_d = nc.dram_tensor("out", [NPART, B_PER, L], f32, kind="ExternalOutput").ap()

    with tile.TileContext(nc) as tc:
        with (
            tc.tile_pool(name="const", bufs=1) as cpool,
            tc.tile_pool(name="psum", bufs=8, space="PSUM") as ppool,
        ):
            meta = cpool.tile([NPART, TOT_META], f32, tag="meta", name="meta")
            dma_chain = [nc.sync.dma_start(out=meta[:, :], in_=meta_d[:, :])]
            # x arrives in serialized chunks (DMA rings drain round-robin, so
            # without the explicit chain every stream finishes last-together
            # and the first matmul waits ~3x longer than needed).  First chunk
            # covers only segment 0's reads (max col 0+14+500=514 -> 520) so
            # PE start is gated by the ~1.6us weight build, not by x.
            XCUTS = (0, 520, 2016, LXP)  # seg0 | segs1-3 | rest
            x_tiles = []
            for b, xd in ((0, xa_d), (1, xb_d)):
                xt = cpool.tile([NPART, LXP], cdt, tag=f"x{b}", name=f"x{b}")
                for lo, hi in zip(XCUTS, XCUTS[1:]):
                    dma_chain.append(
                        nc.sync.dma_start(out=xt[:, lo:hi], in_=xd[:, lo:hi])
                    )
                x_tiles.append(xt)
            # meta and the first x chunk run concurrently (both gate the first
            # matmul; ~2-way bandwidth share still lands them in ~1.4us) —
            # only the later links are serialized behind them.
            for prev, nxt in zip(dma_chain[1:], dma_chain[2:]):
                tile.add_dep_helper(
                    nxt.ins, prev.ins, sync=True, reason="serialize input DMA chain"
                )
            out_t = cpool.tile([NPART, B_PER, L], f32, tag="out", name="out")

            # Build the 8 per-segment block-diagonal stationary matrices
            # [(dk,i) x (f, (dk,o))] on-chip: broadcast the compact weights
            # across the 64 dk column-blocks and multiply by the 0/1 mask.
            w_tiles = []
            wcomp_3d = meta[:, :OFF_MASK].rearrange(
                "p (s f u o) -> p s f u o", s=S, f=NB, u=1, o=CO
            )
            mask_3d = meta[:, OFF_MASK:OFF_BIAS].rearrange(
                "p (u m) -> p u m", u=1
            ).broadcast_to((NPART, NB, NPART))
            mask_2d = meta[:, OFF_MASK:OFF_BIAS].rearrange("p (m o) -> p m o", o=CO)
            for s in range(S):
                wt = cpool.tile([NPART, NB * NPART], cdt, tag=f"w{s}", name=f"w{s}")
                if s == 0:
                    # segment 0 built per-tap: tap 0 lands in ~150ns, so the
                    # first matmul isn't gated on the whole 1920-col build
                    for f in range(NB):
                        base = (s * NB + f) * CO
                        nc.vector.tensor_mul(
                            wt[:, f * NPART : (f + 1) * NPART].rearrange(
                                "p (m o) -> p m o", o=CO
                            ),
                            meta[:, base : base + CO]
                            .rearrange("p (u o) -> p u o", u=1)
                            .broadcast_to((NPART, DK, CO)),
                            mask_2d,
                        )
                else:
                    nc.vector.tensor_mul(
                        wt[:, :].rearrange("p (f m) -> p f m", m=NPART),
                        wcomp_3d[:, s].broadcast_to((NPART, NB, DK, CO)),
                        mask_3d,
                    )
                w_tiles.append(wt)

            def bias_sl(s):
                return meta[:, OFF_BIAS + s : OFF_BIAS + s + 1]

            def body():
                for b in range(B_PER):
                    for s in range(S):
                        for (t0, n_use, n_mm) in _subtiles(s):
                            ps = ppool.tile([NPART, MAX_N], f32, tag="ps", name="ps")
                            for f in range(NB):
                                if tile4:
                                    # 4 diagonal 32x32 subarray matmuls: they
                                    # run concurrently (distinct row+col
                                    # groups) and each weight load is only 32
                                    # columns instead of 128.
                                    for q in range(4):
                                        lo = 32 * q
                                        nc.tensor.matmul(
                                            ps[lo : lo + 32, :n_mm],
                                            lhsT=w_tiles[s][
                                                lo : lo + 32,
                                                f * NPART + lo : f * NPART + lo + 32,
                                            ],
                                            rhs=x_tiles[b][
                                                lo : lo + 32, t0 + f : t0 + f + n_mm
                                            ],
                                            start=(f == 0),
                                            stop=(f == NB - 1),
                                            tile_position=(lo, lo),
                                            skip_group_check=True,
                                        )
                                else:
                                    nc.tensor.matmul(
                                        ps[:, :n_mm],
                                        lhsT=w_tiles[s][:, f * NPART : (f + 1) * NPART],
                                        rhs=x_tiles[b][:, t0 + f : t0 + f + n_mm],
                                        start=(f == 0),
                                        stop=(f == NB - 1),
                                    )
                            nc.vector.tensor_scalar_add(
                                out_t[:, b, t0 : t0 + n_use], ps[:, :n_use], bias_sl(s)
                            )

            if loop_n is not None:
                # PE body is >256 instructions (one IRAM block): hint the
                # back-edge so each iteration doesn't pay a ~4us I$-miss.
                # staggered_reset avoids the ~2us drain+all-engine-barrier
                # back-edge (bench loop only — kernel() never takes this path).
                loop_ctx = tc.For_i(
                    0,
                    loop_n,
                    1,
                    hint_engines=(mybir.EngineType.PE,),
                    staggered_reset=True,
                )
            else:
                loop_ctx = contextlib.nullcontext()
            with loop_ctx:
                for _rep in range(repeat):
                    body()
            # Per-(b, segment) output DMAs: each drains as soon as its
            # segment's bias-add lands, so only the last ~0.26 MB trails the
            # final compute instead of a whole 2.1 MB per-b transfer.
            for b in range(B_PER):
                for s in range(S):
                    t0, ln = SEG_STARTS[s], SEG_LENS[s]
                    nc.sync.dma_start(
                        out=out_d[:, b, t0 : t0 + ln], in_=out_t[:, b, t0 : t0 + ln]
                    )

    nc.compile()
    return nc


def _shard_inputs(x, w, bias, x_dtype=np.float32):
    """Host-side reshape into per-core DRAM layouts."""
    xp = np.pad(x, [(0, 0)] * 4 + [(P, P)])  # [B,D,K,CI,LP]
    in_maps = []
    for core in range(N_CORES):
        dg, bg = divmod(core, BG)
        dsl = slice(dg * D_PER, (dg + 1) * D_PER)
        bsl = slice(bg * B_PER, (bg + 1) * B_PER)

        # x: [B_PER, D_PER, K, CI, LP] -> partitions (d,k,i), cols (b, l),
        # each row zero-extended from LP to LXP
        xs = xp[bsl, dsl]
        x_core = np.zeros((NPART, B_PER, LXP), np.float32)
        x_core[:, :, :LP] = xs.transpose(1, 2, 3, 0, 4).reshape(NPART, B_PER, LP)

        # compact weights: row (dk, i), col (s, f, o) = w[dk, o, i, s, f]
        wd = w[dsl, :, :, :, :, 0, :].reshape(DK, CO, CI, S, NB)
        wcomp = np.ascontiguousarray(
            wd.transpose(0, 2, 3, 4, 1).reshape(NPART, S * NB * CO)
        )

        # block-diag mask
        p = np.arange(NPART)
        mask = (p[:, None] // CO == p[None, :] // CO).astype(np.float32)

        # bias: row (dk, o), col s
        bias_core = np.ascontiguousarray(bias[dsl, :, :, :, 0].reshape(NPART, S))

        meta = np.concatenate([wcomp, mask, bias_core], axis=1)
        in_maps.append(
            {
                "meta": np.ascontiguousarray(meta),
                "xa": np.ascontiguousarray(x_core[:, 0]).astype(x_dtype),
                "xb": np.ascontiguousarray(x_core[:, 1]).astype(x_dtype),
            }
        )
    return in_maps


def _unshard_output(results):
    out = np.empty((B, D, K, CO, L), np.float32)
    for core in range(N_CORES):
        dg, bg = divmod(core, BG)
        oc = results[core]["out"].reshape(D_PER, K, CO, B_PER, L)
        out[bg * B_PER : (bg + 1) * B_PER, dg * D_PER : (dg + 1) * D_PER] = (
            oc.transpose(3, 0, 1, 2, 4)
        )
    return out


def _x_dtype_for(compute_dt):
    if compute_dt == "bfloat16":
        import ml_dtypes

        return ml_dtypes.bfloat16
    if compute_dt == "float16":
        return np.float16
    return np.float32


def run(inputs, trace=False, compute_dt="float32r", tile4=False):
    """Returns (output ndarray, BassKernelResults)."""
    from concourse.bass_utils import run_bass_kernel_spmd

    x = np.asarray(inputs["x"], np.float32)
    w = np.asarray(inputs["weight"], np.float32)
    bias = np.asarray(inputs["bias"], np.float32)

    key = (compute_dt, tile4)
    if key not in _prog_cache:
        _prog_cache[key] = _build_program(compute_dt, tile4=tile4)
    nc = _prog_cache[key]

    in_maps = _shard_inputs(x, w, bias, _x_dtype_for(compute_dt))
    res = run_bass_kernel_spmd(nc, in_maps, list(range(N_CORES)), trace=trace)
    return _unshard_output(res.results), res


def kernel(**inputs) -> np.ndarray:
    out, _ = run(inputs)
    return out


def _build_null_program():
    """Minimal program with the same I/O signature — measures dispatch floor."""
    import concourse.mybir as mybir
    import concourse.tile as tile
    from concourse import bacc

    f32 = mybir.dt.float32
    nc = bacc.Bacc("TRN2", target_bir_lowering=False, debug=False)
    inp_d = nc.dram_tensor("inp", [NPART, TOT_IN], f32, kind="ExternalInput").ap()
    out_d = nc.dram_tensor("out", [NPART, B_PER, L], f32, kind="ExternalOutput").ap()
    with tile.TileContext(nc) as tc:
        with tc.tile_pool(name="t", bufs=1) as pool:
            t = pool.tile([NPART, 8], f32, tag="t", name="t")
            nc.sync.dma_start(out=t[:, :], in_=inp_d[:, :8])
            nc.sync.dma_start(out=out_d[:, 0, :8], in_=t[:, :])
    nc.compile()
    return nc


def _make_callable(nc):
    """One-time jitted shard_map callable for a bass program; zeros for the
    output operands are generated inside the jit (no donation needed)."""
    import jax
    import jax.numpy as jnp
    from jax.experimental.shard_map import shard_map
    from jax.sharding import Mesh, PartitionSpec

    import concourse.mybir as mybir
    from concourse import bass2jax

    bass2jax.install_neuronx_cc_hook()

    partition_name = nc.partition_id_tensor.name if nc.partition_id_tensor else None
    in_names, out_names, out_avals = [], [], []
    for alloc in nc.m.functions[0].allocations:
        if not isinstance(alloc, mybir.MemoryLocationSet):
            continue
        name = alloc.memorylocations[0].name
        if alloc.kind == "ExternalInput":
            if name != partition_name:
                in_names.append(name)
        elif alloc.kind == "ExternalOutput":
            out_names.append(name)
            out_avals.append(
                jax.core.ShapedArray(tuple(alloc.tensor_shape), mybir.dt.np(alloc.dtype))
            )
    n_params = len(in_names)
    all_names = in_names + out_names + ([partition_name] if partition_name else [])

    def _body(*args):
        operands = list(args)
        if partition_name is not None:
            operands.append(bass2jax.partition_id_tensor())
        return tuple(
            bass2jax._bass_exec_p.bind(
                *operands,
                out_avals=tuple(out_avals),
                in_names=tuple(all_names),
                out_names=tuple(out_names),
                lowering_input_output_aliases=(),
                sim_require_finite=True,
                sim_require_nnan=True,
                nc=nc,
            )
        )

    n_outs = len(out_names)
    devices = jax.devices()[:N_CORES]
    mesh = Mesh(np.asarray(devices), ("core",))
    sharding = jax.sharding.NamedSharding(mesh, PartitionSpec("core"))
    jitted = jax.jit(
        shard_map(
            _body,
            mesh=mesh,
            in_specs=(PartitionSpec("core"),) * (n_params + n_outs),
            out_specs=(PartitionSpec("core"),) * n_outs,
            check_rep=False,
        ),
        donate_argnums=tuple(range(n_params, n_params + n_outs)),
        keep_unused=True,
    )

    def _zeros():
        return [
            jax.device_put(
                np.zeros((N_CORES * av.shape[0], *av.shape[1:]), av.dtype), sharding
            )
            for av in out_avals
        ]

    return jitted, in_names, _zeros, sharding


def _time_callable(jitted, args, zeros_fn, warmup=1, iters=5, pause=0.25):
    import time

    import jax

    for _ in range(warmup):
        jax.block_until_ready(jitted(*args, *zeros_fn()))
        time.sleep(pause)
    ts = []
    for _ in range(iters):
        z = zeros_fn()
        jax.block_until_ready(z)
        time.sleep(pause)
        t0 = time.perf_counter()
        jax.block_until_ready(jitted(*args, *z))
        ts.append(time.perf_counter() - t0)
    ts.sort()
    return ts[len(ts) // 2]


def bench(inputs, compute_dt="float32r", n_lo=16, n_hi=216, iters=5, build_kwargs=None):
    """Per-iteration HW time from the slope between two hardware-loop trip
    counts inside single NEFF executions (the ~100 ms axon dispatch floor
    cancels out).  Returns ns per kernel iteration."""
    import jax

    x = np.asarray(inputs["x"], np.float32)
    w = np.asarray(inputs["weight"], np.float32)
    bias = np.asarray(inputs["bias"], np.float32)
    in_maps = _shard_inputs(x, w, bias, _x_dtype_for(compute_dt))

    import time

    bk = dict(build_kwargs or {})
    calls = {}
    concat_in = None
    for n in (n_lo, n_hi):
        key = (compute_dt, "loop", n, tuple(sorted(bk.items())))
        if key not in _prog_cache:
            _prog_cache[key] = _build_program(compute_dt, loop_n=n, **bk)
        jitted, in_names, zeros_fn, sharding = _make_callable(_prog_cache[key])
        if concat_in is None:
            concat_in = [
                jax.device_put(
                    np.concatenate([in_maps[c][nm] for c in range(N_CORES)], axis=0),
                    sharding,
                )
                for nm in in_names
            ]
        calls[n] = (jitted, zeros_fn)

    # Warm both, then interleave lo/hi calls so the slow drift in the per-call
    # dispatch floor cancels in each pairwise difference.
    for n in (n_lo, n_hi):
        jitted, zeros_fn = calls[n]
        jax.block_until_ready(jitted(*concat_in, *zeros_fn()))
        time.sleep(0.2)
    diffs = []
    for _ in range(iters):
        pair = {}
        for n in (n_lo, n_hi):
            jitted, zeros_fn = calls[n]
            z = zeros_fn()
            jax.block_until_ready(z)
            t0 = time.perf_counter()
            jax.block_until_ready(jitted(*concat_in, *z))
            pair[n] = time.perf_counter() - t0
            time.sleep(0.1)
        diffs.append(pair[n_hi] - pair[n_lo])
        print(
            f"  pair: lo {pair[n_lo] * 1e3:.2f} ms  hi {pair[n_hi] * 1e3:.2f} ms"
            f"  diff {(pair[n_hi] - pair[n_lo]) * 1e3:.2f} ms"
        )
    diffs.sort()
    med = diffs[len(diffs) // 2]
    slope_ns = med / (n_hi - n_lo) * 1e9
    print(f"  per-iteration time: {slope_ns:.0f} ns")
    return slope_ns

